# revision 11
# baseline (speedup 1.0000x reference)
"""Trainium2 Bass kernel for nn_DIETModel (multi-hot embedding -> MLP -> 2-layer transformer encoder).

Sharding: data-parallel over batch. 8 cores x 4 batch elements (256 tokens each).
The one-hot scatter + fc1 GEMM is done as an indirect-DMA row gather from
fc1_w.T (plus a zero pad row for dedup padding), summed on-chip, transposed on
the PE into feature-major layout, and the rest of the network runs feature-major
([feature_chunk=128, token=256] tiles) so every linear layer is a plain
lhsT.T @ rhs matmul chain with per-partition bias/activation fusion.

kernel(**inputs) takes the FULL unsharded inputs (same names as
reference.setup_inputs()) and returns the FULL [32, 64, 256] output.
"""

import math
import numpy as np

WORD = 30000
NGRAM = 50000
F = WORD + NGRAM          # 80000 sparse features
B, L, K = 32, 64, 6
D, H, FFD, NL = 256, 8, 512, 2
HD = D // H               # 32
EPS = 1e-5
P = 128
NCORES = 8
BPC = B // NCORES         # batches per core = 4
TOK = BPC * L             # tokens per core = 256
NSLOT = 7                 # word + 6 ngrams (deduped, padded with zero-row index 0)
NLOC = 1800               # per-core local table rows (1 zero row + <=1792 gathered rows)
SCALE = 1.0 / math.sqrt(HD)

# blobA column offsets (bf16)
OFF_FC2 = 0                      # 4 tiles x 256
OFF_QKV = OFF_FC2 + 4 * 256      # 36 tiles x 128 (i,g,k)
OFF_OUT = OFF_QKV + 36 * 128     # 6 tiles x 256 (i,k)
OFF_BM = OFF_OUT + 6 * 256       # 1024
CA = OFF_BM + 1024
# blobB column offsets (bf16)
OFF_FF1 = 0                      # 4 tiles x 1024 (i,k)
OFF_FF2 = OFF_FF1 + 4 * 1024     # 16 tiles x 256 (i,k)
CB = OFF_FF2 + 16 * 256
# bias blob columns (fp32)
BOFF_FC1 = 0       # 4
BOFF_FC2 = 4       # 2
BOFF_QKV = 6       # 2x9
BOFF_OUT = 24      # 2x2
BOFF_FF1 = 28      # 2x8
BOFF_FF2 = 44      # 2x2
BOFF_LNG = 48      # 2x2x2 (i,j,k)
BOFF_LNB = 56      # 2x2x2
_CACHE = {}


def _build_nc():
    import os
    import concourse.bass as bass
    import concourse.mybir as mybir
    import concourse.tile as tile
    from concourse.masks import make_identity
    from contextlib import ExitStack

    fp32 = mybir.dt.float32
    bf = mybir.dt.bfloat16
    i32 = mybir.dt.int32
    AF = mybir.ActivationFunctionType
    ALU = mybir.AluOpType
    AX = mybir.AxisListType

    nc = bass.Bass("TRN2", target_bir_lowering=False, debug=False, num_devices=NCORES)

    # ---------------- DRAM I/O ----------------
    # weights packed column-wise into two bf16 blobs (one DMA each):
    #   blobA = fc2(4x256) | qkv(36x128) | out(6x256) | bmask(1024)
    #   blobB = ff1(4x1024) | ff2(16x256)
    # biases/ln packed into one fp32 blob [128, 64].
    table = nc.dram_tensor("table", [NLOC, FFD], bf, kind="ExternalInput")
    idx = nc.dram_tensor("idx", [P, 2 * NSLOT], i32, kind="ExternalInput")
    blobA = nc.dram_tensor("blobA", [P, CA], bf, kind="ExternalInput")
    blobB = nc.dram_tensor("blobB", [P, CB], bf, kind="ExternalInput")
    bias = nc.dram_tensor("bias", [P, 64], fp32, kind="ExternalInput")
    xT_out = nc.dram_tensor("xT", [2, P, TOK], fp32, kind="ExternalOutput")

    with tile.TileContext(nc, num_cores=NCORES) as tc, ExitStack() as ctx:
        const = ctx.enter_context(tc.tile_pool(name="const", bufs=1))
        wp = ctx.enter_context(tc.tile_pool(name="wp", bufs=1))
        gp = ctx.enter_context(tc.tile_pool(name="gp", bufs=1))
        ap = ctx.enter_context(tc.tile_pool(name="ap", bufs=4))
        # PSUM pools: keep total <= 8 banks (2KB/partition each)
        ps_lin = ctx.enter_context(tc.tile_pool(name="ps_lin", bufs=2, space="PSUM"))
        ps_sc = ctx.enter_context(tc.tile_pool(name="ps_sc", bufs=2, space="PSUM"))
        ps_tr = ctx.enter_context(tc.tile_pool(name="ps_tr", bufs=2, space="PSUM"))
        ps_o = ctx.enter_context(tc.tile_pool(name="ps_o", bufs=1, space="PSUM"))

        # ---------------- constants ----------------
        ident = const.tile([P, P], bf, tag="ident")
        make_identity(nc, ident[:])
        ones_col = const.tile([P, 1], bf, tag="ones_col")
        nc.vector.memset(ones_col[:], 1.0)
        ones_row = const.tile([1, P], bf, tag="ones_row")
        nc.vector.memset(ones_row[:], 1.0)
        eps_t = const.tile([1, 1], fp32, tag="eps")
        nc.vector.memset(eps_t[:], EPS)
        idx_sb = const.tile([P, 2 * NSLOT], i32, tag="idx")
        nc.sync.dma_start(out=idx_sb[:], in_=idx[:])
        bias_sb = wp.tile([P, 64], fp32, tag="bias_sb", name="bias_sb")

        # ---------------- weights to SBUF (2 blob DMAs, issued after the
        # gather DMAs so the gathers get the DMA engines first) ----------------
        wA = wp.tile([P, CA], bf, tag="wA", name="wA")
        wB = wp.tile([P, CB], bf, tag="wB", name="wB")

        def colsA(off, w):
            return wA[:, off : off + w]

        bmask_sb = colsA(OFF_BM, H * P)
        fc1_b_sb = bias_sb[:, BOFF_FC1 : BOFF_FC1 + 4]
        fc2_b_sb = bias_sb[:, BOFF_FC2 : BOFF_FC2 + 2]
        fc2_w_sb = [colsA(OFF_FC2 + k * D, D) for k in range(4)]
        qkv_w_sb = [[[colsA(OFF_QKV + ((i * 9 + g) * 2 + k) * P, P) for k in range(2)] for g in range(9)] for i in range(NL)]
        qkv_b_sb = [bias_sb[:, BOFF_QKV + i * 9 : BOFF_QKV + (i + 1) * 9] for i in range(NL)]
        out_w_sb = [[colsA(OFF_OUT + (i * 3 + k) * D, D) for k in range(3)] for i in range(NL)]
        out_b_sb = [bias_sb[:, BOFF_OUT + i * 2 : BOFF_OUT + (i + 1) * 2] for i in range(NL)]
        ff1_w_sb = [[wB[:, OFF_FF1 + (i * 2 + k) * 4 * D : OFF_FF1 + (i * 2 + k + 1) * 4 * D] for k in range(2)] for i in range(NL)]
        ff1_b_sb = [bias_sb[:, BOFF_FF1 + i * 8 : BOFF_FF1 + (i + 1) * 8] for i in range(NL)]
        ff2_w_sb = [[wB[:, OFF_FF2 + (i * 8 + k) * D : OFF_FF2 + (i * 8 + k + 1) * D] for k in range(8)] for i in range(NL)]
        ff2_b_sb = [bias_sb[:, BOFF_FF2 + i * 2 : BOFF_FF2 + (i + 1) * 2] for i in range(NL)]
        ln_g_sb = [[bias_sb[:, BOFF_LNG + (i * 2 + j) * 2 : BOFF_LNG + (i * 2 + j + 1) * 2] for j in range(2)] for i in range(NL)]
        ln_b_sb = [[bias_sb[:, BOFF_LNB + (i * 2 + j) * 2 : BOFF_LNB + (i * 2 + j + 1) * 2] for j in range(2)] for i in range(NL)]

        # ---------------- fc1: gather + sum + transpose + relu ----------------
        # gather: for slot j, token chunk tch: rows table[idx[p, j*2+tch]] -> [128, 512]
        gtiles = [[None, None] for _ in range(NSLOT)]
        for j in range(NSLOT):
            for tch in range(2):
                g = gp.tile([P, FFD], bf, tag=f"g{j}_{tch}", name=f"g{j}_{tch}")
                nc.gpsimd.indirect_dma_start(
                    out=g[:],
                    out_offset=None,
                    in_=table[:, :],
                    in_offset=bass.IndirectOffsetOnAxis(ap=idx_sb[:, j * 2 + tch : j * 2 + tch + 1], axis=0),
                )
                gtiles[j][tch] = g
        nc.sync.dma_start(out=bias_sb[:], in_=bias[:])
        nc.sync.dma_start(out=wA[:], in_=blobA[:])
        nc.sync.dma_start(out=wB[:], in_=blobB[:])

        # tree-sum the 7 slots (token-major [128 tok, 512 feat]) per chunk
        sum_t = []
        for tch in range(2):
            eng = nc.vector
            acc = ap.tile([P, FFD], bf, tag=f"fc1sum{tch}", name=f"fc1sum{tch}", bufs=1)
            eng.tensor_add(out=acc[:], in0=gtiles[0][tch][:], in1=gtiles[1][tch][:])
            for j in range(2, NSLOT):
                eng.tensor_add(out=acc[:], in0=acc[:], in1=gtiles[j][tch][:])
            sum_t.append(acc)

        # transpose to feature-major + relu(x + b)
        # (one matmul/transpose per PSUM tile: two matmul groups writing one
        # PSUM bank at different offsets is a hardware fault)
        x1r = []
        for f in range(4):
            t = ap.tile([P, TOK], bf, tag="x1r", name=f"x1r{f}", bufs=4)
            for tch in range(2):
                pt = ps_tr.tile([P, P], bf, tag="tr", name=f"x1t{f}_{tch}")
                nc.tensor.transpose(
                    out=pt[:],
                    in_=sum_t[tch][:, f * P : (f + 1) * P],
                    identity=ident[:],
                )
                nc.scalar.activation(out=t[:, tch * P : (tch + 1) * P], in_=pt[:], func=AF.Relu,
                                     bias=fc1_b_sb[:, f : f + 1], scale=1.0)
            x1r.append(t)

        # ---------------- fc2 -> residual stream x (feature-major, 2 tiles) ----------------
        x = []
        for m in range(2):
            pl = ps_lin.tile([P, TOK], fp32, tag="lin", name=f"fc2p{m}")
            for k in range(4):
                nc.tensor.matmul(
                    out=pl[:],
                    lhsT=fc2_w_sb[k][:, m * P : (m + 1) * P],
                    rhs=x1r[k][:],
                    start=(k == 0),
                    stop=(k == 3),
                )
            t = ap.tile([P, TOK], bf, tag="x", name=f"x0_{m}", bufs=6)
            nc.scalar.activation(out=t[:], in_=pl[:], func=AF.Identity, bias=fc2_b_sb[:, m : m + 1], scale=1.0)
            x.append(t)

        # ---------------- layer norm helper (feature-major) ----------------
        def layer_norm(xin, g_sb, b_sb, li, which, out_dt=None):
            out_dt = out_dt or bf
            # stats: sum(x) and sum(x^2) over all 256 features (separate PSUM
            # tiles: one matmul group per bank)
            sx = ps_tr.tile([1, TOK], fp32, tag="tr", name=f"lnsx{li}_{which}")
            for k in range(2):
                nc.tensor.matmul(out=sx[0:1, :], lhsT=ones_col[:, 0:1], rhs=xin[k][:],
                                 start=(k == 0), stop=(k == 1))
            xsq = []
            for k in range(2):
                t = ap.tile([P, TOK], bf, tag="xsq", name=f"xsq{li}_{which}_{k}")
                nc.vector.tensor_mul(out=t[:], in0=xin[k][:], in1=xin[k][:])
                xsq.append(t)
            sxx = ps_tr.tile([1, TOK], fp32, tag="tr", name=f"lnsxx{li}_{which}")
            for k in range(2):
                nc.tensor.matmul(out=sxx[0:1, :], lhsT=ones_col[:, 0:1], rhs=xsq[k][:],
                                 start=(k == 0), stop=(k == 1))
            # moments -> a = rstd, c = mean * rstd  (all [1, 256])
            s_sb = ap.tile([1, 2 * TOK], fp32, tag="lns", name=f"lns{li}_{which}")
            nc.scalar.mul(out=s_sb[:, 0:TOK], in_=sx[0:1, :], mul=1.0 / D)
            nc.scalar.mul(out=s_sb[:, TOK : 2 * TOK], in_=sxx[0:1, :], mul=1.0 / D)
            t1 = ap.tile([1, TOK], fp32, tag="lnt", name=f"lnt{li}_{which}")
            nc.vector.tensor_mul(out=t1[:], in0=s_sb[:, 0:TOK], in1=s_sb[:, 0:TOK])
            nc.vector.tensor_tensor(out=t1[:], in0=s_sb[:, TOK : 2 * TOK], in1=t1[:], op=ALU.subtract)
            nc.scalar.activation(out=t1[:], in_=t1[:], func=AF.Sqrt, bias=eps_t[0:1, 0:1])
            acf = ap.tile([1, 2 * TOK], fp32, tag="lnacf", name=f"lnacf{li}_{which}")
            nc.vector.reciprocal(out=acf[:, 0:TOK], in_=t1[:])
            nc.vector.tensor_mul(out=acf[:, TOK : 2 * TOK], in0=s_sb[:, 0:TOK], in1=acf[:, 0:TOK])
            ac = ap.tile([1, 2 * TOK], bf, tag="lnac", name=f"lnac{li}_{which}")
            nc.vector.tensor_copy(out=ac[:], in_=acf[:])
            # broadcast a|c across partitions via ones-column matmul
            bc = ps_lin.tile([P, 2 * TOK], fp32, tag="lin", name=f"lnbc{li}_{which}")
            nc.tensor.matmul(out=bc[:], lhsT=ones_row[0:1, :], rhs=ac[:], start=True, stop=True)
            outt = []
            for k in range(2):
                engl = nc.vector
                t2 = ap.tile([P, TOK], bf, tag="lnapp", name=f"lnapp{li}_{which}_{k}")
                engl.tensor_mul(out=t2[:], in0=xin[k][:], in1=bc[:, 0:TOK])
                engl.tensor_tensor(out=t2[:], in0=t2[:], in1=bc[:, TOK : 2 * TOK], op=ALU.subtract)
                t3 = ap.tile([P, TOK], out_dt, tag="x", name=f"ln{li}_{which}_{k}", bufs=6)
                engl.tensor_scalar(out=t3[:], in0=t2[:], scalar1=g_sb[:, k : k + 1],
                                   scalar2=b_sb[:, k : k + 1], op0=ALU.mult, op1=ALU.add)
                outt.append(t3)
            return outt

        stage = int(os.environ.get("KERNEL_STAGE", "99"))

        # head h -> (group t, partition offset off); groups hold <=3 heads so
        # off is always 0/32/64
        def hmap(h):
            return (h // 3, (h % 3) * 32) if h < 6 else (2, (h - 6) * 32)

        GW = [96, 96, 64]  # rows used per head group

        # ---------------- transformer layers ----------------
        for i in range(NL):
            if stage <= 1 + 2 * i:
                break
            # qkv projection: 9 head-group tiles (g = section*3 + t), rows 0:GW[t]
            qkvT = []
            for g in range(9):
                pl = ps_lin.tile([P, TOK], fp32, tag="lin", name=f"qkvp{i}_{g}")
                for k in range(2):
                    nc.tensor.matmul(out=pl[:], lhsT=qkv_w_sb[i][g][k][:],
                                     rhs=x[k][:], start=(k == 0), stop=(k == 1))
                t = ap.tile([P, TOK], bf, tag="qkv", name=f"qkvT{i}_{g}", bufs=10)
                if g % 2 == 1:
                    nc.vector.tensor_scalar_add(out=t[:], in0=pl[:], scalar1=qkv_b_sb[i][:, g : g + 1])
                else:
                    nc.scalar.activation(out=t[:], in_=pl[:], func=AF.Identity,
                                         bias=qkv_b_sb[i][:, g : g + 1], scale=1.0)
                qkvT.append(t)
            if stage == 11:
                x = [qkvT[0], qkvT[3]]
                break

            # scores + softmax (exp(scale*s) masked, per-head rowsum normalize)
            Pn = []  # per tch: [128, 1024] normalized probs (q tokens on partitions)
            for tch in range(2):
                E = ap.tile([P, H * P], bf, tag="E", name=f"E{i}_{tch}")
                for h in range(H):
                    t_, off = hmap(h)
                    sc = ps_sc.tile([P, P], fp32, tag="sc", name=f"sc{i}_{tch}_{h}")
                    nc.tensor.matmul(
                        out=sc[:],
                        lhsT=qkvT[t_][off : off + 32, tch * P : (tch + 1) * P],
                        rhs=qkvT[3 + t_][off : off + 32, tch * P : (tch + 1) * P],
                        start=True, stop=True,
                    )
                    nc.scalar.activation(out=E[:, h * P : (h + 1) * P], in_=sc[:],
                                         func=AF.Exp, scale=SCALE)
                enge = nc.vector
                enge.tensor_mul(out=E[:], in0=E[:], in1=bmask_sb[:])
                rs = ap.tile([P, H], fp32, tag="rs", name=f"rs{i}_{tch}")
                nc.vector.reduce_sum(out=rs[:], in_=E[:].rearrange("p (h k) -> p h k", h=H), axis=AX.X)
                rcp = ap.tile([P, H], fp32, tag="rcp", name=f"rcp{i}_{tch}")
                nc.vector.reciprocal(out=rcp[:], in_=rs[:])
                for h in range(H):
                    enge.tensor_scalar_mul(out=E[:, h * P : (h + 1) * P], in0=E[:, h * P : (h + 1) * P],
                                           scalar1=rcp[:, h : h + 1])
                Pn.append(E)
            if stage == 12:
                x = [Pn[0], Pn[1]]
                break

            # v transposed to token-major: vtok[h] [128 (ktok), 64] (cols tch*32+hd)
            vtok = []
            for h in range(H):
                t_, off = hmap(h)
                t = ap.tile([P, 64], bf, tag="vtok", name=f"vtok{i}_{h}", bufs=8)
                for tch in range(2):
                    vt_ps = ps_tr.tile([P, 32], bf, tag="tr", name=f"vt{i}_{h}_{tch}")
                    nc.tensor.transpose(
                        out=vt_ps[:],
                        in_=qkvT[6 + t_][off : off + 32, tch * P : (tch + 1) * P],
                        identity=ident[off : off + 32, off : off + 32],
                    )
                    if h % 2 == 0:
                        nc.vector.tensor_copy(out=t[:, tch * 32 : (tch + 1) * 32], in_=vt_ps[:])
                    else:
                        nc.scalar.copy(out=t[:, tch * 32 : (tch + 1) * 32], in_=vt_ps[:])
                vtok.append(t)

            # attn transpose PT[h] [128 (ktok), 256 (qtok)], then o per
            # (head-group, chunk) in its own PSUM tile (heads write disjoint
            # partition ranges; out_w columns are host-permuted to match the
            # head-group row order)
            PTs = []
            for h in range(H):
                PT = ap.tile([P, TOK], bf, tag="PT", name=f"PT{i}_{h}", bufs=8)
                for tch in range(2):
                    pt_ps = ps_tr.tile([P, P], bf, tag="tr", name=f"pt{i}_{h}_{tch}")
                    nc.tensor.transpose(out=pt_ps[:], in_=Pn[tch][:, h * P : (h + 1) * P], identity=ident[:])
                    if h % 2 == 0:
                        nc.vector.tensor_copy(out=PT[:, tch * P : (tch + 1) * P], in_=pt_ps[:])
                    else:
                        nc.scalar.copy(out=PT[:, tch * P : (tch + 1) * P], in_=pt_ps[:])
                PTs.append(PT)
            GRPS = [[0, 1, 2], [3, 4, 5], [6, 7]]
            o_sb = []
            for g in range(3):
                t = ap.tile([P, TOK], bf, tag="osb", name=f"osb{i}_{g}", bufs=3)
                if GW[g] < P:
                    nc.vector.memset(t[:], 0.0)
                for tch in range(2):
                    o_ps = ps_o.tile([P, P], fp32, tag="o", name=f"o{i}_{g}_{tch}")
                    for h in GRPS[g]:
                        _, off = hmap(h)
                        nc.tensor.matmul(
                            out=o_ps[off : off + 32, :],
                            lhsT=vtok[h][:, tch * 32 : (tch + 1) * 32],
                            rhs=PTs[h][:, tch * P : (tch + 1) * P],
                            start=True, stop=True,
                        )
                    if g % 2 == 0:
                        nc.vector.tensor_copy(out=t[0 : GW[g], tch * P : (tch + 1) * P],
                                              in_=o_ps[0 : GW[g], :])
                    else:
                        nc.scalar.copy(out=t[0 : GW[g], tch * P : (tch + 1) * P],
                                       in_=o_ps[0 : GW[g], :])
                o_sb.append(t)
            if stage == 13:
                x = [o_sb[0], o_sb[1]]
                break

            # out projection + residual
            xa = []
            for m in range(2):
                pl = ps_lin.tile([P, TOK], fp32, tag="lin", name=f"outp{i}_{m}")
                for k in range(3):
                    nc.tensor.matmul(out=pl[:], lhsT=out_w_sb[i][k][:, m * P : (m + 1) * P],
                                     rhs=o_sb[k][:], start=(k == 0), stop=(k == 2))
                t = ap.tile([P, TOK], bf, tag="xa", name=f"xa{i}_{m}")
                nc.scalar.activation(out=t[:], in_=pl[:], func=AF.Identity,
                                     bias=out_b_sb[i][:, m : m + 1], scale=1.0)
                t2 = ap.tile([P, TOK], bf, tag="xar", name=f"xar{i}_{m}")
                nc.vector.tensor_add(out=t2[:], in0=t[:], in1=x[m][:])
                xa.append(t2)

            x = layer_norm(xa, ln_g_sb[i][0], ln_b_sb[i][0], i, 0)

            if stage <= 2 + 2 * i:
                continue

            # feed-forward
            f_sb = []
            for m in range(8):
                pl = ps_lin.tile([P, TOK], fp32, tag="lin", name=f"ff1p{i}_{m}")
                for k in range(2):
                    nc.tensor.matmul(out=pl[:], lhsT=ff1_w_sb[i][k][:, m * P : (m + 1) * P],
                                     rhs=x[k][:], start=(k == 0), stop=(k == 1))
                t = ap.tile([P, TOK], bf, tag="fsb", name=f"fsb{i}_{m}", bufs=8)
                if m % 2 == 1:
                    nc.vector.tensor_scalar(out=t[:], in0=pl[:], scalar1=ff1_b_sb[i][:, m : m + 1],
                                            scalar2=0.0, op0=ALU.add, op1=ALU.max)
                else:
                    nc.scalar.activation(out=t[:], in_=pl[:], func=AF.Relu,
                                         bias=ff1_b_sb[i][:, m : m + 1], scale=1.0)
                f_sb.append(t)
            xf = []
            for m in range(2):
                pl = ps_lin.tile([P, TOK], fp32, tag="lin", name=f"ff2p{i}_{m}")
                for k in range(8):
                    nc.tensor.matmul(out=pl[:], lhsT=ff2_w_sb[i][k][:, m * P : (m + 1) * P],
                                     rhs=f_sb[k][:], start=(k == 0), stop=(k == 7))
                t = ap.tile([P, TOK], bf, tag="xf", name=f"xf{i}_{m}")
                nc.scalar.activation(out=t[:], in_=pl[:], func=AF.Identity,
                                     bias=ff2_b_sb[i][:, m : m + 1], scale=1.0)
                t2 = ap.tile([P, TOK], bf, tag="xfr", name=f"xfr{i}_{m}")
                nc.vector.tensor_add(out=t2[:], in0=t[:], in1=x[m][:])
                xf.append(t2)

            x = layer_norm(xf, ln_g_sb[i][1], ln_b_sb[i][1], i, 1, out_dt=(fp32 if i == NL - 1 else None))

        # ---------------- output ----------------
        for c in range(2):
            nc.sync.dma_start(out=xT_out[c], in_=x[c][:, 0:TOK])

    return nc


def _split_excess_waits(nc, max_waits=1):
    """walrus setupSyncWait rejects >1 sem wait on CTRL-encoded instructions.
    Move excess waits onto wait-only Drain instructions inserted immediately
    before the offender on the same engine (per-engine streams are in-order,
    so sequential waits are equivalent to combined waits)."""
    import concourse.mybir as mybir

    ctr = 0
    for fn in nc.m.functions:
        for bb in fn.blocks:
            insts = bb.instructions
            new, changed = [], False
            for inst in insts:
                si = inst.sync_info
                if si is not None and len(si.on_wait) > max_waits:
                    waits = list(si.on_wait)
                    extra, keep = waits[:-max_waits], waits[-max_waits:]
                    for i in range(0, len(extra), max_waits):
                        d = mybir.InstDrain(name=f"wsplit-{ctr}", ins=[], outs=[])
                        ctr += 1
                        d.engine = inst.engine
                        d.sync_info = mybir.SyncInfo(on_wait=extra[i : i + max_waits], on_update=[])
                        new.append(d)
                        changed = True
                    si.on_wait = keep
                new.append(inst)
            if changed:
                bb.instructions = new
    return nc


def _prep_host_inputs(inputs):
    """Transpose/chunk all weights into the kernel's DRAM layouts (shared
    across cores) and build per-core index tensors."""
    f32 = np.float32
    g = {k: np.asarray(v) for k, v in inputs.items()}

    tableT = np.ascontiguousarray(g["fc1_w"].astype(f32).T)  # [80000, 512]

    def chunkT(w):  # [Dout, Din] -> [Din/128, 128, Dout]
        wT = np.ascontiguousarray(w.astype(f32).T)
        return np.ascontiguousarray(wT.reshape(wT.shape[0] // P, P, wT.shape[1]))

    def biasT(b):  # [Dout] -> [128, Dout/128]
        b = np.asarray(b, f32)
        return np.ascontiguousarray(b.reshape(-1, P).T)

    # head groups of <=3 heads -> per-head 32-row slices at partition 0/32/64
    HGRP = [[0, 1, 2], [3, 4, 5], [6, 7]]

    def qkv_perm(i):  # -> [9, 2, 128, 128] (cols past the group's heads are zero)
        wT = np.ascontiguousarray(g["qkv_w"][i].astype(f32).T)  # [256 in, 768 out]
        tiles = []
        for sec in range(3):
            for t in range(3):
                cols = [sec * D + h * HD + j for h in HGRP[t] for j in range(HD)]
                slab = np.zeros((D, P), f32)
                slab[:, : len(cols)] = wT[:, cols]
                tiles.append(slab.reshape(2, P, P))
        return np.stack(tiles)

    def qkv_bias_perm(i):  # -> [128, 9]
        b = np.asarray(g["qkv_b"][i], f32)
        out = np.zeros((P, 9), f32)
        for sec in range(3):
            for t in range(3):
                cols = [sec * D + h * HD + j for h in HGRP[t] for j in range(HD)]
                out[: len(cols), sec * 3 + t] = b[cols]
        return out

    def outw_perm(i):  # -> [3, 128, 256] with K rows permuted to head-group order
        wT = np.ascontiguousarray(g["out_w"][i].astype(f32).T)  # [256 (o feat), 256 out]
        tiles = []
        for t in range(3):
            rows = [h * HD + j for h in HGRP[t] for j in range(HD)]
            slab = np.zeros((P, D), f32)
            slab[: len(rows)] = wT[rows]
            tiles.append(slab)
        return np.stack(tiles)

    import ml_dtypes
    bf16 = ml_dtypes.bfloat16

    # block-diagonal batch mask, repeated per head: [128, 8*128]
    bm = np.zeros((P, P), f32)
    bm[:64, :64] = 1.0
    bm[64:, 64:] = 1.0
    bmask = np.ascontiguousarray(np.tile(bm, (1, H)))

    # blobA: fc2 | qkv | out | bmask  (bf16, column-packed)
    partsA = [np.hstack(list(chunkT(g["fc2_w"])))]
    for i in range(NL):
        qp = qkv_perm(i)  # [9, 2, 128, 128]
        partsA.extend(qp[gg, k] for gg in range(9) for k in range(2))
    for i in range(NL):
        op = outw_perm(i)  # [3, 128, 256]
        partsA.extend(op[k] for k in range(3))
    partsA.append(bmask)
    blobA = np.ascontiguousarray(np.hstack(partsA)).astype(bf16)
    assert blobA.shape == (P, CA), blobA.shape

    partsB = []
    for i in range(NL):
        partsB.extend(list(chunkT(g["ff1_w"][i])))
    for i in range(NL):
        partsB.extend(list(chunkT(g["ff2_w"][i])))
    blobB = np.ascontiguousarray(np.hstack(partsB)).astype(bf16)
    assert blobB.shape == (P, CB), blobB.shape

    bias = np.zeros((P, 64), f32)
    bias[:, BOFF_FC1:BOFF_FC1 + 4] = biasT(g["fc1_b"])
    bias[:, BOFF_FC2:BOFF_FC2 + 2] = biasT(g["fc2_b"])
    for i in range(NL):
        bias[:, BOFF_QKV + i * 9:BOFF_QKV + (i + 1) * 9] = qkv_bias_perm(i)
        bias[:, BOFF_OUT + i * 2:BOFF_OUT + (i + 1) * 2] = biasT(g["out_b"][i])
        bias[:, BOFF_FF1 + i * 8:BOFF_FF1 + (i + 1) * 8] = biasT(g["ff1_b"][i])
        bias[:, BOFF_FF2 + i * 2:BOFF_FF2 + (i + 1) * 2] = biasT(g["ff2_b"][i])
        bias[:, BOFF_LNG + i * 4:BOFF_LNG + i * 4 + 2] = biasT(g["ln1_g"][i])
        bias[:, BOFF_LNG + i * 4 + 2:BOFF_LNG + i * 4 + 4] = biasT(g["ln2_g"][i])
        bias[:, BOFF_LNB + i * 4:BOFF_LNB + i * 4 + 2] = biasT(g["ln1_b"][i])
        bias[:, BOFF_LNB + i * 4 + 2:BOFF_LNB + i * 4 + 4] = biasT(g["ln2_b"][i])

    shared = {"blobA": blobA, "blobB": blobB, "bias": bias}

    # indices: word + offset ngrams, dedup within token (multi-hot .set semantics),
    # pad with sentinel -1 (-> local zero row)
    word = g["word_idx"].astype(np.int64).reshape(B * L, 1)
    ngr = g["ngram_idx"].astype(np.int64).reshape(B * L, K) + WORD
    arr = np.concatenate([word, ngr], axis=1)
    arr.sort(axis=1)
    dup = arr[:, 1:] == arr[:, :-1]
    arr[:, 1:][dup] = -1  # [B*L, 7] global rows, -1 = pad

    in_maps = []
    for c in range(NCORES):
        idx_c = arr[c * TOK : (c + 1) * TOK]  # [256, 7]
        # shard the table: this core's local table holds only the rows its
        # tokens reference (row 0 stays zero for dedupe padding); the kernel
        # still performs the real indirect-DMA gather on device.
        uniq = np.unique(idx_c[idx_c >= 0])
        assert len(uniq) + 1 <= NLOC
        loc_table = np.zeros((NLOC, FFD), bf16)
        loc_table[1 : 1 + len(uniq)] = tableT[uniq].astype(bf16)
        remap = np.zeros(F, np.int32)
        remap[uniq] = np.arange(1, 1 + len(uniq), dtype=np.int32)
        idx_loc = np.where(idx_c >= 0, remap[np.clip(idx_c, 0, F - 1)], 0).astype(np.int32)
        idx_sb = np.empty((P, 2 * NSLOT), np.int32)
        for j in range(NSLOT):
            for tch in range(2):
                idx_sb[:, j * 2 + tch] = idx_loc[tch * P : (tch + 1) * P, j]
        m = dict(shared)
        m["table"] = loc_table
        m["idx"] = idx_sb
        in_maps.append(m)
    return in_maps


LAST_RESULTS = None


def kernel(**inputs):
    global LAST_RESULTS
    from concourse.bass_utils import run_bass_kernel_spmd

    if "nc" not in _CACHE:
        _CACHE["nc"] = _split_excess_waits(_build_nc())
    nc = _CACHE["nc"]

    in_maps = _prep_host_inputs(inputs)
    res = run_bass_kernel_spmd(nc, in_maps, list(range(NCORES)))
    LAST_RESULTS = res

    out = np.empty((B, L, D), np.float32)
    for c in range(NCORES):
        xT = res.results[c]["xT"].reshape(2 * P, TOK)  # [256 feat, 256 tok]
        out[c * BPC : (c + 1) * BPC] = xT.T.reshape(BPC, L, D)
    return out

